# revision 4
# baseline (speedup 1.0000x reference)
"""Trainium2 Bass kernel for nn_Block_36438502540029 (involution CNN block).

Structure per core (data-parallel over batch, 2 images/core):
  conv1 (1x1, 512->128) -> ReLU -> padded bf16 buffer
  reduce (1x1, 128->32, M-replicated x4) -> ReLU -> w1rep
  span+broadcast: per-tap stationary Ws_dup[32,128] (rows duplicated 16x
    host-side), row-tiled pair matmuls -> per-pixel kernel maps in PSUM
  apply: ACT drains PSUM->SBUF bf16 (drained pairs) or DVE reads PSUM
    directly (direct pairs); DVE bf16 muls vs shifted padded x1;
    PE identity-matmul accumulation into out2 PSUM
  conv3 (1x1, 128->512) + identity residual via extra matmul + bias -> out

All matmuls bf16 (full rate); PSUM accumulation fp32.
"""
import numpy as np
import ml_dtypes
from contextlib import ExitStack

import concourse.bass as bass
import concourse.tile as tile
from concourse import bacc, mybir
from concourse import bass_utils

bf16 = mybir.dt.bfloat16
f32 = mybir.dt.float32
AF = mybir.ActivationFunctionType
ALU = mybir.AluOpType
BF = ml_dtypes.bfloat16

N_CORES = 8
B, CIN, H, W = 16, 512, 28, 28
BL = B // N_CORES            # images per core
CMID, CRED, G, GCH = 128, 32, 8, 16
KS, PD = 7, 3                # kernel size, pad
HWPX = H * W                 # 784
NPX = BL * HWPX              # 1568
PW = W + 2 * PD              # 34
PIMG = PW * PW               # 1156
NPAD = BL * PIMG + 8         # 2320 (slack for strided quarter views)
NTAP = KS * KS               # 49
NPAIR = (NTAP + 1) // 2      # 25 (last pair single)

CHUNKS = [(0, 512), (512, 512), (1024, 512), (1536, 32)]
NQ = 4                       # spatial quarters of 392
QW = NPX // NQ               # 392

# pairs whose taps use the direct-from-PSUM DVE path (no ACT drain)
DIRECT_PAIRS = frozenset({2, 5, 8, 11, 14, 17, 20, 23})
# taps whose accumulation runs on DVE instead of PE (tune for balance)
DVE_ACCUM_TAPS = frozenset()

_prog_cache = {}


def _build_program(direct_pairs=DIRECT_PAIRS):
    nc = bacc.Bacc("TRN2", num_devices=N_CORES, debug=False)

    dr = {}
    dr["x"] = nc.dram_tensor("x", [128, 4 * NPX], bf16, kind="ExternalInput")
    dr["w1t"] = nc.dram_tensor("w1t", [128, 512], bf16, kind="ExternalInput")
    dr["wrt"] = nc.dram_tensor("wrt", [128, 128], bf16, kind="ExternalInput")
    dr["wsd"] = nc.dram_tensor("wsd", [64, NPAIR * 128], bf16, kind="ExternalInput")
    dr["w3t"] = nc.dram_tensor("w3t", [128, 512], bf16, kind="ExternalInput")
    dr["ident"] = nc.dram_tensor("ident", [128, 128], bf16, kind="ExternalInput")
    dr["b1"] = nc.dram_tensor("b1", [128, 1], f32, kind="ExternalInput")
    dr["brr"] = nc.dram_tensor("brr", [128, 1], f32, kind="ExternalInput")
    dr["bsd"] = nc.dram_tensor("bsd", [128, NTAP], f32, kind="ExternalInput")
    dr["b3"] = nc.dram_tensor("b3", [128, 4], f32, kind="ExternalInput")
    y = nc.dram_tensor("y", [128, 4 * NPX], bf16, kind="ExternalOutput")

    with tile.TileContext(nc) as tc:
        with ExitStack() as ctx:
            const = ctx.enter_context(tc.tile_pool(name="const", bufs=1))
            sbuf = ctx.enter_context(tc.tile_pool(name="sbuf", bufs=1))
            wsmp = ctx.enter_context(tc.tile_pool(name="wsm", bufs=3))
            prodp = ctx.enter_context(tc.tile_pool(name="prod", bufs=4))
            ystg = ctx.enter_context(tc.tile_pool(name="ystg", bufs=2))
            psA = ctx.enter_context(tc.tile_pool(name="psA", bufs=1, space="PSUM"))
            psB = ctx.enter_context(tc.tile_pool(name="psB", bufs=2, space="PSUM"))

            # ---- constant loads ----
            w1t_sb = const.tile([128, 512], bf16, name="w1t_sb")
            nc.sync.dma_start(w1t_sb[:], dr["w1t"].ap())
            wrt_sb = const.tile([128, 128], bf16, name="wrt_sb")
            nc.sync.dma_start(wrt_sb[:], dr["wrt"].ap())
            wsd_sb = const.tile([128, NPAIR * 128], bf16, name="wsd_sb")
            nc.sync.dma_start(wsd_sb[0:64, :], dr["wsd"].ap())
            w3t_sb = const.tile([128, 512], bf16, name="w3t_sb")
            nc.sync.dma_start(w3t_sb[:], dr["w3t"].ap())
            id_sb = const.tile([128, 128], bf16, name="id_sb")
            nc.sync.dma_start(id_sb[:], dr["ident"].ap())
            b1_sb = const.tile([128, 1], f32, name="b1_sb")
            nc.sync.dma_start(b1_sb[:], dr["b1"].ap())
            brr_sb = const.tile([128, 1], f32, name="brr_sb")
            nc.sync.dma_start(brr_sb[:], dr["brr"].ap())
            bsd_sb = const.tile([128, NTAP], f32, name="bsd_sb")
            nc.sync.dma_start(bsd_sb[:], dr["bsd"].ap())
            b3_sb = const.tile([128, 4], f32, name="b3_sb")
            nc.sync.dma_start(b3_sb[:], dr["b3"].ap())

            xsb = sbuf.tile([128, 4 * NPX], bf16, name="xsb")
            for k in range(4):
                nc.sync.dma_start(xsb[:, NPX * k:NPX * (k + 1)],
                                  dr["x"].ap()[:, NPX * k:NPX * (k + 1)])

            pad_t = sbuf.tile([128, NPAD], bf16, name="pad_t")
            nc.vector.memset(pad_t[:], 0.0)
            pad4 = pad_t[:, 0:BL * PIMG].rearrange("p (b i j) -> p b i j", b=BL, i=PW, j=PW)

            # ---- conv1: out1 = relu(W1' @ x + b1), K-outer so compute
            # starts as soon as the first x tile lands ----
            c1ps = psA.tile([128, NPX], f32, tag="big", name="c1ps")
            for k in range(4):
                for (off, wd) in CHUNKS:
                    nc.tensor.matmul(
                        c1ps[:, off:off + wd],
                        w1t_sb[:, 128 * k:128 * (k + 1)],
                        xsb[:, NPX * k + off:NPX * k + off + wd],
                        start=(k == 0), stop=(k == 3),
                    )
            nc.scalar.activation(
                pad4[:, :, PD:PD + H, PD:PD + W],
                c1ps[:].rearrange("p (b i j) -> p b i j", b=BL, i=H, j=W),
                AF.Relu, bias=b1_sb[:], scale=1.0,
            )

            # ---- reduce: w1rep = relu(Wr'_rep @ out1 + br_rep) ----
            redps = psA.tile([128, 2048], f32, tag="big", name="redps")
            for q in range(NQ):
                b_, hh = q // 2, q % 2
                rhs = pad4[:, b_:b_ + 1, PD + 14 * hh:PD + 14 * hh + 14, PD:PD + W]
                nc.tensor.matmul(redps[:, 512 * q:512 * q + QW], wrt_sb[:], rhs,
                                 start=True, stop=True)
            w1rep = sbuf.tile([128, NPX], bf16, name="w1rep")
            nc.scalar.activation(
                w1rep[:].rearrange("p (a n) -> p a n", a=4, n=QW),
                redps[:].rearrange("p (a n) -> p a n", a=4, n=512)[:, :, 0:QW],
                AF.Relu, bias=brr_sb[:], scale=1.0,
            )

            # ---- involution apply ----
            out2ps = psA.tile([128, NPX], f32, tag="big", name="out2ps")

            def pad_shift(t, q=None, squeeze=False):
                di, dj = t // KS - PD, t % KS - PD
                if q is None:
                    return pad4[:, :, PD + di:PD + di + H, PD + dj:PD + dj + W]
                b_, hh = q // 2, q % 2
                r0 = PD + di + 14 * hh
                if squeeze:
                    off = b_ * PIMG + r0 * PW + PD + dj
                    return pad_t[:, off:off + 14 * PW].rearrange(
                        "p (i j) -> p i j", i=14, j=PW)[:, :, 0:W]
                return pad4[:, b_:b_ + 1, r0:r0 + 14, PD + dj:PD + dj + W]

            for p in range(NPAIR):
                taps = [t for t in (2 * p, 2 * p + 1) if t < NTAP]
                ns = len(taps)
                drained = p not in direct_pairs
                wm = None
                prods = []
                for s, t in enumerate(taps):
                    pr = prodp.tile([128, NPX], bf16, tag="prod", name=f"prod{t}")
                    prods.append(pr)
                if drained:
                    wm = wsmp.tile([128, ns * NPX], bf16, tag="wm", name=f"wm{p}")
                for q in range(NQ):
                    bq = psB.tile([128, 1024], f32, tag="bc", name=f"bc{p}_{q}")
                    for s, t in enumerate(taps):
                        nc.tensor.matmul(
                            bq[:, 512 * s:512 * s + QW],
                            wsd_sb[32 * s:32 * (s + 1), 128 * p:128 * (p + 1)],
                            w1rep[32 * s:32 * (s + 1), QW * q:QW * (q + 1)],
                            start=True, stop=True,
                            tile_position=(32 * s, 0),
                        )
                    if drained:
                        nc.scalar.activation(
                            wm[:].rearrange("p (s n) -> p s n", s=ns, n=NPX)[
                                :, :, QW * q:QW * (q + 1)],
                            bq[:].rearrange("p (s n) -> p s n", s=2, n=512)[
                                :, 0:ns, 0:QW],
                            AF.Identity, bias=0.0, scale=1.0,
                        )
                    else:
                        for s, t in enumerate(taps):
                            nc.vector.scalar_tensor_tensor(
                                prods[s][:, QW * q:QW * (q + 1)].rearrange(
                                    "p (i j) -> p i j", i=14, j=W),
                                bq[:, 512 * s:512 * s + QW].rearrange(
                                    "p (i j) -> p i j", i=14, j=W),
                                bsd_sb[:, t:t + 1],
                                pad_shift(t, q, squeeze=True),
                                ALU.add, ALU.mult,
                            )
                if drained:
                    for s, t in enumerate(taps):
                        nc.vector.tensor_mul(
                            prods[s][:].rearrange("p (b i j) -> p b i j",
                                                  b=BL, i=H, j=W),
                            wm[:, NPX * s:NPX * (s + 1)].rearrange(
                                "p (b i j) -> p b i j", b=BL, i=H, j=W),
                            pad_shift(t),
                        )
                for s, t in enumerate(taps):
                    for (off, wd) in CHUNKS:
                        nc.tensor.matmul(
                            out2ps[:, off:off + wd], id_sb[:],
                            prods[s][:, off:off + wd],
                            start=(t == 0), stop=(t == NTAP - 1),
                            skip_group_check=True,
                        )

            out2sb = sbuf.tile([128, NPX], bf16, name="out2sb")
            nc.scalar.copy(out2sb[:], out2ps[:])

            # ---- conv3 + residual: y_m = W3'_m @ out2 + x_m + b3_m ----
            for m in range(4):
                for hh in range(2):
                    c3 = psB.tile([128, 1024], f32, tag="bc", name=f"c3_{m}_{hh}")
                    hoff = HWPX * hh
                    for (off, wd) in [(0, 512), (512, 272)]:
                        nc.tensor.matmul(
                            c3[:, off:off + wd], w3t_sb[:, 128 * m:128 * (m + 1)],
                            out2sb[:, hoff + off:hoff + off + wd],
                            start=True, stop=False, skip_group_check=True,
                        )
                    for (off, wd) in [(0, 512), (512, 272)]:
                        nc.tensor.matmul(
                            c3[:, off:off + wd], id_sb[:],
                            xsb[:, NPX * m + hoff + off:NPX * m + hoff + off + wd],
                            start=False, stop=True, skip_group_check=True,
                        )
                    ysb = ystg.tile([128, HWPX], bf16, tag="y", name=f"y{m}_{hh}")
                    nc.scalar.activation(ysb[:], c3[:, 0:HWPX], AF.Identity,
                                         bias=b3_sb[:, m:m + 1], scale=1.0)
                    nc.sync.dma_start(y.ap()[:, NPX * m + hoff:NPX * m + hoff + HWPX],
                                      ysb[:])

    nc.compile()
    return nc


def get_program(all_direct=False):
    key = "nc_all_direct" if all_direct else "nc"
    if key not in _prog_cache:
        dp = frozenset(range(NPAIR)) if all_direct else DIRECT_PAIRS
        _prog_cache[key] = _build_program(dp)
    return _prog_cache[key]


def _host_prep(inputs):
    """Fold scales into weights; build per-core DRAM tensor layouts."""
    x = np.asarray(inputs["x"], np.float32)
    W1 = np.asarray(inputs["W1"], np.float32) * np.asarray(inputs["s1"], np.float32)[:, None]
    Wr = np.asarray(inputs["Wr"], np.float32) * np.asarray(inputs["sr"], np.float32)[:, None]
    Ws = np.asarray(inputs["Ws"], np.float32)
    W3 = np.asarray(inputs["W3"], np.float32) * np.asarray(inputs["s3"], np.float32)[:, None]
    b1 = np.asarray(inputs["b1"], np.float32)
    br = np.asarray(inputs["br"], np.float32)
    bs = np.asarray(inputs["bs"], np.float32)
    b3 = np.asarray(inputs["b3"], np.float32)

    w1t = np.ascontiguousarray(
        W1.T.reshape(4, 128, 128).transpose(1, 0, 2).reshape(128, 512)).astype(BF)
    wrt = np.tile(Wr.T, (1, 4)).astype(BF)
    # wsd[32*s + j, 128*p + g*16 + ch] = Ws[g*49 + (2p+s), j]
    wsd = np.zeros((64, NPAIR * 128), np.float32)
    WsT = Ws.reshape(G, NTAP, CRED)  # [g, t, j]
    for p in range(NPAIR):
        for s in range(2):
            t = 2 * p + s
            if t >= NTAP:
                continue
            blk = WsT[:, t, :].T  # [j, g]
            wsd[32 * s:32 * s + 32, 128 * p:128 * (p + 1)] = np.repeat(
                blk, GCH, axis=1)  # [j, g*16+ch]
    wsd = wsd.astype(BF)
    w3t = W3.T.astype(BF)  # [128, 512], col = 128m+mm
    ident = np.eye(128, dtype=np.float32).astype(BF)
    bsd = np.repeat(bs.reshape(G, NTAP), GCH, axis=0)  # [128, 49]
    bsd = np.ascontiguousarray(bsd).astype(np.float32)

    base = {
        "w1t": w1t, "wrt": wrt, "wsd": wsd, "w3t": w3t, "ident": ident,
        "b1": b1.reshape(128, 1).astype(np.float32),
        "brr": np.tile(br, 4).reshape(128, 1).astype(np.float32),
        "bsd": bsd,
        "b3": np.ascontiguousarray(b3.reshape(4, 128).T).astype(np.float32),
    }
    in_maps = []
    for c in range(N_CORES):
        xs = x[BL * c:BL * (c + 1)]  # [2, 512, 28, 28]
        xc = np.ascontiguousarray(
            xs.reshape(BL, 4, 128, HWPX).transpose(2, 1, 0, 3).reshape(128, 4 * NPX)
        ).astype(BF)
        m = dict(base)
        m["x"] = xc
        in_maps.append(m)
    return in_maps


def _unshard(results):
    out = np.empty((B, CIN, H, W), np.float32)
    for c in range(N_CORES):
        yc = results[c]["y"].astype(np.float32)  # [128, 6272]
        # col = 1568*m + 784*b + pix ; channel = 128*m + partition
        yv = yc.reshape(128, 4, BL, H, W).transpose(2, 1, 0, 3, 4)  # [b, m, pp, h, w]
        out[BL * c:BL * (c + 1)] = yv.reshape(BL, CIN, H, W)
    return out


def kernel(**inputs):
    # the fast drained path folds bs into nothing (bs==0 in this problem's
    # setup_inputs); nonzero bs routes every pair through the direct path,
    # which applies bs exactly
    all_direct = bool(np.abs(np.asarray(inputs["bs"])).max() > 0)
    nc = get_program(all_direct)
    in_maps = _host_prep(inputs)
    import os
    trace = bool(os.environ.get("KERNEL_TRACE"))
    kw = {}
    if trace:
        import tempfile
        kw = dict(trace=True, tmpdir=tempfile.mkdtemp(prefix="ktr_"))
        try:
            import ntff_shim  # noqa: F401
        except ImportError:
            pass
    res = bass_utils.run_bass_kernel_spmd(
        nc, in_maps, core_ids=list(range(N_CORES)), **kw)
    if trace and res.exec_time_ns is not None:
        prof = os.environ.get("KERNEL_PROFILE_OUT")
        if prof:
            with open(prof, "w") as f:
                f.write(str(res.exec_time_ns))
        print(f"HW exec time: {res.exec_time_ns} ns")
    return _unshard(res.results)


# revision 5
# speedup vs baseline: 1.1883x; 1.1883x over previous
"""Trainium2 Bass kernel for nn_Block_36438502540029 (involution CNN block).

Structure per core (data-parallel over batch, 2 images/core):
  conv1 (1x1, 512->128) -> ReLU -> padded bf16 buffer
  reduce (1x1, 128->32, M-replicated x4) -> ReLU -> w1rep
  span+broadcast: per-tap stationary Ws_dup[32,128] (rows duplicated 16x
    host-side), row-tiled pair matmuls -> per-pixel kernel maps in PSUM
  apply: ACT drains PSUM->SBUF bf16 (drained pairs) or DVE reads PSUM
    directly (direct pairs); DVE bf16 muls vs shifted padded x1;
    PE identity-matmul accumulation into out2 PSUM
  conv3 (1x1, 128->512) + identity residual via extra matmul + bias -> out

All matmuls bf16 (full rate); PSUM accumulation fp32.
"""
import numpy as np
import ml_dtypes
from contextlib import ExitStack

import concourse.bass as bass
import concourse.tile as tile
from concourse import bacc, mybir
from concourse import bass_utils

bf16 = mybir.dt.bfloat16
f32 = mybir.dt.float32
AF = mybir.ActivationFunctionType
ALU = mybir.AluOpType
BF = ml_dtypes.bfloat16

N_CORES = 8
B, CIN, H, W = 16, 512, 28, 28
BL = B // N_CORES            # images per core
CMID, CRED, G, GCH = 128, 32, 8, 16
KS, PD = 7, 3                # kernel size, pad
HWPX = H * W                 # 784
NPX = BL * HWPX              # 1568
PW = W + 2 * PD              # 34
PIMG = PW * PW               # 1156
NPAD = BL * PIMG + 8         # 2320 (slack for strided quarter views)
NTAP = KS * KS               # 49
NPAIR = (NTAP + 1) // 2      # 25 (last pair single)

CHUNKS = [(0, 512), (512, 512), (1024, 512), (1536, 32)]
NQ = 4                       # spatial quarters of 392
QW = NPX // NQ               # 392

# pairs whose taps use the direct-from-PSUM DVE path (no ACT drain)
DIRECT_PAIRS = frozenset({2, 5, 8, 11, 14, 17, 20, 23})
# taps whose accumulation runs on DVE instead of PE (tune for balance)
DVE_ACCUM_TAPS = frozenset()

_prog_cache = {}


def _build_program(direct_pairs=DIRECT_PAIRS):
    nc = bacc.Bacc("TRN2", num_devices=N_CORES, debug=False)

    dr = {}
    dr["x"] = nc.dram_tensor("x", [128, 4 * NPX], bf16, kind="ExternalInput")
    dr["w1t"] = nc.dram_tensor("w1t", [128, 512], bf16, kind="ExternalInput")
    dr["wrt"] = nc.dram_tensor("wrt", [128, 128], bf16, kind="ExternalInput")
    dr["wsd"] = nc.dram_tensor("wsd", [64, NPAIR * 128], bf16, kind="ExternalInput")
    dr["w3t"] = nc.dram_tensor("w3t", [128, 512], bf16, kind="ExternalInput")
    dr["ident"] = nc.dram_tensor("ident", [128, 128], bf16, kind="ExternalInput")
    dr["b1"] = nc.dram_tensor("b1", [128, 1], f32, kind="ExternalInput")
    dr["brr"] = nc.dram_tensor("brr", [128, 1], f32, kind="ExternalInput")
    dr["bsd"] = nc.dram_tensor("bsd", [128, NTAP], f32, kind="ExternalInput")
    dr["b3"] = nc.dram_tensor("b3", [128, 4], f32, kind="ExternalInput")
    y = nc.dram_tensor("y", [128, 4 * NPX], bf16, kind="ExternalOutput")

    with tile.TileContext(nc) as tc:
        with ExitStack() as ctx:
            const = ctx.enter_context(tc.tile_pool(name="const", bufs=1))
            sbuf = ctx.enter_context(tc.tile_pool(name="sbuf", bufs=1))
            wsmp = ctx.enter_context(tc.tile_pool(name="wsm", bufs=3))
            prodp = ctx.enter_context(tc.tile_pool(name="prod", bufs=6))
            ystg = ctx.enter_context(tc.tile_pool(name="ystg", bufs=2))
            psA = ctx.enter_context(tc.tile_pool(name="psA", bufs=1, space="PSUM"))
            psB = ctx.enter_context(tc.tile_pool(name="psB", bufs=2, space="PSUM"))

            # ---- constant loads ----
            w1t_sb = const.tile([128, 512], bf16, name="w1t_sb")
            nc.sync.dma_start(w1t_sb[:], dr["w1t"].ap())
            wrt_sb = const.tile([128, 128], bf16, name="wrt_sb")
            nc.sync.dma_start(wrt_sb[:], dr["wrt"].ap())
            wsd_sb = const.tile([128, NPAIR * 128], bf16, name="wsd_sb")
            nc.sync.dma_start(wsd_sb[0:64, :], dr["wsd"].ap())
            w3t_sb = const.tile([128, 512], bf16, name="w3t_sb")
            nc.sync.dma_start(w3t_sb[:], dr["w3t"].ap())
            id_sb = const.tile([128, 128], bf16, name="id_sb")
            nc.sync.dma_start(id_sb[:], dr["ident"].ap())
            b1_sb = const.tile([128, 1], f32, name="b1_sb")
            nc.sync.dma_start(b1_sb[:], dr["b1"].ap())
            brr_sb = const.tile([128, 1], f32, name="brr_sb")
            nc.sync.dma_start(brr_sb[:], dr["brr"].ap())
            bsd_sb = const.tile([128, NTAP], f32, name="bsd_sb")
            nc.sync.dma_start(bsd_sb[:], dr["bsd"].ap())
            b3_sb = const.tile([128, 4], f32, name="b3_sb")
            nc.sync.dma_start(b3_sb[:], dr["b3"].ap())

            xsb = sbuf.tile([128, 4 * NPX], bf16, name="xsb")
            for k in range(4):
                nc.sync.dma_start(xsb[:, NPX * k:NPX * (k + 1)],
                                  dr["x"].ap()[:, NPX * k:NPX * (k + 1)])

            pad_t = sbuf.tile([128, NPAD], bf16, name="pad_t")
            nc.vector.memset(pad_t[:], 0.0)
            pad4 = pad_t[:, 0:BL * PIMG].rearrange("p (b i j) -> p b i j", b=BL, i=PW, j=PW)

            # ---- conv1: out1 = relu(W1' @ x + b1), K-outer so compute
            # starts as soon as the first x tile lands ----
            c1ps = psA.tile([128, NPX], f32, tag="big", name="c1ps")
            for k in range(4):
                for (off, wd) in CHUNKS:
                    nc.tensor.matmul(
                        c1ps[:, off:off + wd],
                        w1t_sb[:, 128 * k:128 * (k + 1)],
                        xsb[:, NPX * k + off:NPX * k + off + wd],
                        start=(k == 0), stop=(k == 3),
                    )
            nc.scalar.activation(
                pad4[:, :, PD:PD + H, PD:PD + W],
                c1ps[:].rearrange("p (b i j) -> p b i j", b=BL, i=H, j=W),
                AF.Relu, bias=b1_sb[:], scale=1.0,
            )

            # ---- reduce: w1rep = relu(Wr'_rep @ out1 + br_rep) ----
            redps = psA.tile([128, 2048], f32, tag="big", name="redps")
            for q in range(NQ):
                b_, hh = q // 2, q % 2
                rhs = pad4[:, b_:b_ + 1, PD + 14 * hh:PD + 14 * hh + 14, PD:PD + W]
                nc.tensor.matmul(redps[:, 512 * q:512 * q + QW], wrt_sb[:], rhs,
                                 start=True, stop=True)
            w1rep = sbuf.tile([128, NPX], bf16, name="w1rep")
            nc.scalar.activation(
                w1rep[:].rearrange("p (a n) -> p a n", a=4, n=QW),
                redps[:].rearrange("p (a n) -> p a n", a=4, n=512)[:, :, 0:QW],
                AF.Relu, bias=brr_sb[:], scale=1.0,
            )

            # ---- involution apply ----
            out2ps = psA.tile([128, NPX], f32, tag="big", name="out2ps")

            def pad_shift(t, q=None, squeeze=False):
                di, dj = t // KS - PD, t % KS - PD
                if q is None:
                    return pad4[:, :, PD + di:PD + di + H, PD + dj:PD + dj + W]
                b_, hh = q // 2, q % 2
                r0 = PD + di + 14 * hh
                if squeeze:
                    off = b_ * PIMG + r0 * PW + PD + dj
                    return pad_t[:, off:off + 14 * PW].rearrange(
                        "p (i j) -> p i j", i=14, j=PW)[:, :, 0:W]
                return pad4[:, b_:b_ + 1, r0:r0 + 14, PD + dj:PD + dj + W]

            # software pipeline: accumulate pair p-1 while broadcasting pair
            # p, so the PE stream never stalls on drains/muls (keeps HAM warm)
            pending = []

            def emit_accum(prods_taps):
                for pr, t in prods_taps:
                    for (off, wd) in CHUNKS:
                        nc.tensor.matmul(
                            out2ps[:, off:off + wd], id_sb[:],
                            pr[:, off:off + wd],
                            start=(t == 0), stop=(t == NTAP - 1),
                            skip_group_check=True,
                        )

            for p in range(NPAIR):
                taps = [t for t in (2 * p, 2 * p + 1) if t < NTAP]
                ns = len(taps)
                drained = p not in direct_pairs
                wm = None
                prods = []
                for s, t in enumerate(taps):
                    pr = prodp.tile([128, NPX], bf16, tag="prod", name=f"prod{t}")
                    prods.append(pr)
                if drained:
                    wm = wsmp.tile([128, ns * NPX], bf16, tag="wm", name=f"wm{p}")
                for q in range(NQ):
                    bq = psB.tile([128, 1024], f32, tag="bc", name=f"bc{p}_{q}")
                    for s, t in enumerate(taps):
                        nc.tensor.matmul(
                            bq[:, 512 * s:512 * s + QW],
                            wsd_sb[32 * s:32 * (s + 1), 128 * p:128 * (p + 1)],
                            w1rep[32 * s:32 * (s + 1), QW * q:QW * (q + 1)],
                            start=True, stop=True,
                            tile_position=(32 * s, 0),
                        )
                    if drained:
                        nc.scalar.activation(
                            wm[:].rearrange("p (s n) -> p s n", s=ns, n=NPX)[
                                :, :, QW * q:QW * (q + 1)],
                            bq[:].rearrange("p (s n) -> p s n", s=2, n=512)[
                                :, 0:ns, 0:QW],
                            AF.Identity, bias=0.0, scale=1.0,
                        )
                    else:
                        for s, t in enumerate(taps):
                            nc.vector.scalar_tensor_tensor(
                                prods[s][:, QW * q:QW * (q + 1)].rearrange(
                                    "p (i j) -> p i j", i=14, j=W),
                                bq[:, 512 * s:512 * s + QW].rearrange(
                                    "p (i j) -> p i j", i=14, j=W),
                                bsd_sb[:, t:t + 1],
                                pad_shift(t, q, squeeze=True),
                                ALU.add, ALU.mult,
                            )
                if drained:
                    for s, t in enumerate(taps):
                        nc.vector.tensor_mul(
                            prods[s][:].rearrange("p (b i j) -> p b i j",
                                                  b=BL, i=H, j=W),
                            wm[:, NPX * s:NPX * (s + 1)].rearrange(
                                "p (b i j) -> p b i j", b=BL, i=H, j=W),
                            pad_shift(t),
                        )
                pending.append([(prods[s], t) for s, t in enumerate(taps)])
                if len(pending) > 1:
                    emit_accum(pending.pop(0))
            while pending:
                emit_accum(pending.pop(0))

            out2sb = sbuf.tile([128, NPX], bf16, name="out2sb")
            nc.scalar.copy(out2sb[:], out2ps[:])

            # ---- conv3 + residual: y_m = W3'_m @ out2 + x_m + b3_m ----
            for m in range(4):
                for hh in range(2):
                    c3 = psB.tile([128, 1024], f32, tag="bc", name=f"c3_{m}_{hh}")
                    hoff = HWPX * hh
                    for (off, wd) in [(0, 512), (512, 272)]:
                        nc.tensor.matmul(
                            c3[:, off:off + wd], w3t_sb[:, 128 * m:128 * (m + 1)],
                            out2sb[:, hoff + off:hoff + off + wd],
                            start=True, stop=False, skip_group_check=True,
                        )
                    for (off, wd) in [(0, 512), (512, 272)]:
                        nc.tensor.matmul(
                            c3[:, off:off + wd], id_sb[:],
                            xsb[:, NPX * m + hoff + off:NPX * m + hoff + off + wd],
                            start=False, stop=True, skip_group_check=True,
                        )
                    ysb = ystg.tile([128, HWPX], bf16, tag="y", name=f"y{m}_{hh}")
                    nc.scalar.activation(ysb[:], c3[:, 0:HWPX], AF.Identity,
                                         bias=b3_sb[:, m:m + 1], scale=1.0)
                    nc.sync.dma_start(y.ap()[:, NPX * m + hoff:NPX * m + hoff + HWPX],
                                      ysb[:])

    nc.compile()
    return nc


def get_program(all_direct=False):
    key = "nc_all_direct" if all_direct else "nc"
    if key not in _prog_cache:
        dp = frozenset(range(NPAIR)) if all_direct else DIRECT_PAIRS
        _prog_cache[key] = _build_program(dp)
    return _prog_cache[key]


def _host_prep(inputs):
    """Fold scales into weights; build per-core DRAM tensor layouts."""
    x = np.asarray(inputs["x"], np.float32)
    W1 = np.asarray(inputs["W1"], np.float32) * np.asarray(inputs["s1"], np.float32)[:, None]
    Wr = np.asarray(inputs["Wr"], np.float32) * np.asarray(inputs["sr"], np.float32)[:, None]
    Ws = np.asarray(inputs["Ws"], np.float32)
    W3 = np.asarray(inputs["W3"], np.float32) * np.asarray(inputs["s3"], np.float32)[:, None]
    b1 = np.asarray(inputs["b1"], np.float32)
    br = np.asarray(inputs["br"], np.float32)
    bs = np.asarray(inputs["bs"], np.float32)
    b3 = np.asarray(inputs["b3"], np.float32)

    w1t = np.ascontiguousarray(
        W1.T.reshape(4, 128, 128).transpose(1, 0, 2).reshape(128, 512)).astype(BF)
    wrt = np.tile(Wr.T, (1, 4)).astype(BF)
    # wsd[32*s + j, 128*p + g*16 + ch] = Ws[g*49 + (2p+s), j]
    wsd = np.zeros((64, NPAIR * 128), np.float32)
    WsT = Ws.reshape(G, NTAP, CRED)  # [g, t, j]
    for p in range(NPAIR):
        for s in range(2):
            t = 2 * p + s
            if t >= NTAP:
                continue
            blk = WsT[:, t, :].T  # [j, g]
            wsd[32 * s:32 * s + 32, 128 * p:128 * (p + 1)] = np.repeat(
                blk, GCH, axis=1)  # [j, g*16+ch]
    wsd = wsd.astype(BF)
    w3t = W3.T.astype(BF)  # [128, 512], col = 128m+mm
    ident = np.eye(128, dtype=np.float32).astype(BF)
    bsd = np.repeat(bs.reshape(G, NTAP), GCH, axis=0)  # [128, 49]
    bsd = np.ascontiguousarray(bsd).astype(np.float32)

    base = {
        "w1t": w1t, "wrt": wrt, "wsd": wsd, "w3t": w3t, "ident": ident,
        "b1": b1.reshape(128, 1).astype(np.float32),
        "brr": np.tile(br, 4).reshape(128, 1).astype(np.float32),
        "bsd": bsd,
        "b3": np.ascontiguousarray(b3.reshape(4, 128).T).astype(np.float32),
    }
    in_maps = []
    for c in range(N_CORES):
        xs = x[BL * c:BL * (c + 1)]  # [2, 512, 28, 28]
        xc = np.ascontiguousarray(
            xs.reshape(BL, 4, 128, HWPX).transpose(2, 1, 0, 3).reshape(128, 4 * NPX)
        ).astype(BF)
        m = dict(base)
        m["x"] = xc
        in_maps.append(m)
    return in_maps


def _unshard(results):
    out = np.empty((B, CIN, H, W), np.float32)
    for c in range(N_CORES):
        yc = results[c]["y"].astype(np.float32)  # [128, 6272]
        # col = 1568*m + 784*b + pix ; channel = 128*m + partition
        yv = yc.reshape(128, 4, BL, H, W).transpose(2, 1, 0, 3, 4)  # [b, m, pp, h, w]
        out[BL * c:BL * (c + 1)] = yv.reshape(BL, CIN, H, W)
    return out


def kernel(**inputs):
    # the fast drained path folds bs into nothing (bs==0 in this problem's
    # setup_inputs); nonzero bs routes every pair through the direct path,
    # which applies bs exactly
    all_direct = bool(np.abs(np.asarray(inputs["bs"])).max() > 0)
    nc = get_program(all_direct)
    in_maps = _host_prep(inputs)
    import os
    trace = bool(os.environ.get("KERNEL_TRACE"))
    kw = {}
    if trace:
        import tempfile
        kw = dict(trace=True, tmpdir=tempfile.mkdtemp(prefix="ktr_"))
        try:
            import ntff_shim  # noqa: F401
        except ImportError:
            pass
    res = bass_utils.run_bass_kernel_spmd(
        nc, in_maps, core_ids=list(range(N_CORES)), **kw)
    if trace and res.exec_time_ns is not None:
        prof = os.environ.get("KERNEL_PROFILE_OUT")
        if prof:
            with open(prof, "w") as f:
                f.write(str(res.exec_time_ns))
        print(f"HW exec time: {res.exec_time_ns} ns")
    return _unshard(res.results)
